# revision 1
# baseline (speedup 1.0000x reference)
"""Trainium2 Bass kernel for nn_CaptionModel (GRU + Bahdanau attention caption decoder).

Sharding: pure data-parallel over batch. B=64 -> 8 cores x 8 rows each; no
collectives (50 sequential steps cannot afford the ~5us/call collective floor).

Per-core plan (feature-major: features on partitions, local batch b=8 on free):
  setup:  enc = W_feat.T @ spatialT (+b_feat)        [512, 392]
          att1 = W_ea.T @ enc (+b_ea)                [256, 392] bf16
          enc_bd: block-diag [l, d] pair tiles for the context matmuls
          gi_emb = W_ih[:, :EMB].T @ embT (+biases)  [1536, 400] f32
  50 steps (weight-stationary matmuls, bf16 weights):
          gh   = W_hh.T @ h            (48 mm)
          att2 = W_da.T @ h (+b_da)    (8 mm)
          tanh(att1 + att2) -> scores = tanhT @ W_fa  (per-b mm into [l-part, b-col] psum)
          exp -> denom (ones mm) -> 1/denom -> broadcast (ones outer-product mm)
          context (block-diag mm) -> x_ctx = ctx * rinv
          gi_ctx = W_ihc.T @ x_ctx     (48 mm, accumulated with gh in psum for r,z)
          gates: sigmoid via 0.5+0.5*tanh(x/2) (single ACT table set: exp+tanh)
          h stored fp32; bf16 copy appended to H_hist
  tail:   logits = H_hist.T @ W_fc + b_fc, batch-major out, streamed to DRAM.

kernel() accepts FULL inputs, does host-side layout prep/sharding (incl. the
embedding-table gather), runs the same NEFF SPMD on cores 0-7, unshards.
"""

import contextlib

import ml_dtypes
import numpy as np

import concourse.bass as bass
import concourse.mybir as mybir
from concourse import bacc
from concourse.alu_op_type import AluOpType as Op
from concourse.masks import make_identity
from concourse.tile import TileContext

AF = mybir.ActivationFunctionType
F32 = mybir.dt.float32
BF16 = mybir.dt.bfloat16
F32R = mybir.dt.float32r

B, L, ENC, DEC, EMB, ATT, V, T = 64, 49, 2048, 512, 512, 256, 10000, 50
NCORES = 8
BL = B // NCORES          # 8 local batch rows
NL = BL * L               # 392
KE = ENC // 128           # 16 K-chunks for enc matmul
KD = DEC // 128           # 4 K-chunks over DEC
MG = (3 * DEC) // 128     # 12 M-chunks over gates
MA = ATT // 128           # 2 M-chunks over ATT
NPAIR = BL // 2           # 4 block-diag pairs
HCOL = 8 * (T + 1)        # 408 cols per chunk in H history
NV = 512                  # fc vocab tile width
NCK = (V + NV - 1) // NV  # 20 fc vocab tiles (last = 272 wide)


def build_program(n_steps=T, do_rec=True, do_fc=True):
    nc = bacc.Bacc()
    NT = BL * n_steps      # t*8+b columns
    hcol = 8 * (n_steps + 1)

    # ---------------- DRAM I/O (per-core, host-prepped layouts) ----------------
    d_spatialT = nc.dram_tensor("spatialT", [128, KE * NL], BF16, kind="ExternalInput")
    d_embT = nc.dram_tensor("embT", [128, KD * NT], BF16, kind="ExternalInput")
    d_wfeat = nc.dram_tensor("wfeat", [128, KE * DEC], BF16, kind="ExternalInput")
    d_wea = nc.dram_tensor("wea", [128, KD * ATT], BF16, kind="ExternalInput")
    d_wihe = nc.dram_tensor("wihe", [128, KD * 3 * DEC], BF16, kind="ExternalInput")
    d_wihc = nc.dram_tensor("wihc", [128, KD * MG * 128], BF16, kind="ExternalInput")
    d_whh = nc.dram_tensor("whh", [128, KD * MG * 128], BF16, kind="ExternalInput")
    d_wda = nc.dram_tensor("wda", [128, KD * MA * 128], BF16, kind="ExternalInput")
    d_wfa = nc.dram_tensor("wfa", [128, MA], BF16, kind="ExternalInput")
    d_wfc = nc.dram_tensor("wfc", [128, KD * V], BF16, kind="ExternalInput")
    d_bfeat = nc.dram_tensor("bfeat", [128, KD], F32, kind="ExternalInput")
    d_bea = nc.dram_tensor("bea", [128, MA], F32, kind="ExternalInput")
    d_biasgi = nc.dram_tensor("biasgi", [128, MG], F32, kind="ExternalInput")
    d_bhhnbc = nc.dram_tensor("bhhnbc", [128, 4 * BL], BF16, kind="ExternalInput")
    d_bfc = nc.dram_tensor("bfc", [1, V], BF16, kind="ExternalInput")
    d_logits = nc.dram_tensor("logits", [NT, V], BF16, kind="ExternalOutput")

    with TileContext(nc) as tc, contextlib.ExitStack() as ctx:
        const = ctx.enter_context(tc.tile_pool(name="const", bufs=1))
        state = ctx.enter_context(tc.tile_pool(name="state", bufs=1))

        # persistent weights / constants in SBUF
        wihc = const.tile([128, KD * MG * 128], BF16, tag="wihc")
        whh = const.tile([128, KD * MG * 128], BF16, tag="whh")
        wda = const.tile([128, KD * MA * 128], BF16, tag="wda")
        wfa = const.tile([128, MA], BF16, tag="wfa")
        bhhnbc = const.tile([128, 4 * BL], BF16, tag="bhhnbc")
        biasgi = const.tile([128, MG], F32, tag="biasgi")
        ident_f = const.tile([128, 128], BF16, tag="ident_f")
        make_identity(nc, ident_f[:])
        ones_mat_bf = const.tile([128, 128], BF16, tag="ones_mat")
        bfcb = const.tile([128, V], BF16, tag="bfcb")
        wfc_sb = const.tile([128, KD * V], BF16, tag="wfc_sb")
        for dst, src in [(wihc, d_wihc), (whh, d_whh), (wda, d_wda), (wfa, d_wfa),
                         (bhhnbc, d_bhhnbc), (biasgi, d_biasgi)]:
            nc.sync.dma_start(dst[:], src[:])
        nc.gpsimd.memset(ones_mat_bf[:], 1.0)
        nc.sync.dma_start(bfcb[:], d_bfc[:].partition_broadcast(128))

        # persistent activations / state
        att1 = state.tile([128, MA * NL], BF16, tag="att1")
        enc_bd = [state.tile([128, KD * 128], BF16, tag=f"encbd{j}", name=f"encbd{j}") for j in range(NPAIR)]
        gi_emb = state.tile([128, MG * NT], BF16, tag="gi_emb")
        hist = state.tile([128, KD * hcol], BF16, tag="hist")
        expe = state.tile([128, BL], BF16, tag="expe")
        tanh_sb = state.tile([128, MA * BL * 64], BF16, tag="tanh_sb")
        nc.gpsimd.memset(tanh_sb[:], 0.0)
        nc.gpsimd.memset(hist[:], 0.0)
        nc.gpsimd.memset(expe[:], 0.0)
        for j in range(NPAIR):
            nc.gpsimd.memset(enc_bd[j][:], 0.0)

        # ------------------------------ setup phase ------------------------------
        with tc.tile_pool(name="ssb", bufs=1) as ssb, \
             tc.tile_pool(name="sps", bufs=2, space="PSUM") as sps:
            spatialT = ssb.tile([128, KE * NL], BF16, tag="spatialT")
            embT = ssb.tile([128, KD * NT], BF16, tag="embT")
            wfeat = ssb.tile([128, KE * DEC], BF16, tag="wfeat")
            wea = ssb.tile([128, KD * ATT], BF16, tag="wea")
            wihe = ssb.tile([128, KD * 3 * DEC], BF16, tag="wihe")
            bfeat = ssb.tile([128, KD], F32, tag="bfeat")
            bea = ssb.tile([128, MA], F32, tag="bea")
            for dst, src in [(spatialT, d_spatialT), (embT, d_embT), (wfeat, d_wfeat),
                             (wea, d_wea), (wihe, d_wihe), (bfeat, d_bfeat), (bea, d_bea)]:
                nc.sync.dma_start(dst[:], src[:])

            enc_fm = ssb.tile([128, KD * NL], BF16, tag="enc_fm")
            # enc = W_feat.T @ spatialT  (+ b_feat), feature-major [dec-chunk, b*49+l]
            for mc in range(KD):
                p = sps.tile([128, NL], F32, tag="p_enc")
                for kc in range(KE):
                    nc.tensor.matmul(
                        p[:],
                        wfeat[:, kc * DEC + mc * 128: kc * DEC + mc * 128 + 128],
                        spatialT[:, kc * NL: (kc + 1) * NL],
                        start=(kc == 0), stop=(kc == KE - 1))
                nc.vector.tensor_scalar(
                    enc_fm[:, mc * NL: (mc + 1) * NL], p[:],
                    bfeat[:, mc: mc + 1], None, Op.add)

            # att1 = W_ea.T @ enc (+ b_ea)  -> bf16 [att-chunk, b*49+l]
            for mc in range(MA):
                p = sps.tile([128, NL], F32, tag="p_att1")
                for kc in range(KD):
                    nc.tensor.matmul(
                        p[:],
                        wea[:, kc * ATT + mc * 128: kc * ATT + mc * 128 + 128],
                        enc_fm[:, kc * NL: (kc + 1) * NL],
                        start=(kc == 0), stop=(kc == KD - 1))
                nc.vector.tensor_scalar(
                    att1[:, mc * NL: (mc + 1) * NL], p[:],
                    bea[:, mc: mc + 1], None, Op.add)

            # enc_bd[j]: rows 0:49 = enc[b=2j] (l, d); rows 64:113 = enc[b=2j+1]
            # (psum/ACT partition bases must be 0/32/64 -> 64-offset pairing).
            for c in range(KD):
                for b in range(BL):
                    base = 64 * (b % 2)
                    pt = sps.tile([128, 128], F32, tag="p_tr")
                    nc.tensor.matmul(
                        pt[base: base + L, :],
                        enc_fm[:, c * NL + b * L: c * NL + b * L + L],
                        ident_f[:], start=True, stop=True)
                    nc.vector.tensor_copy(
                        enc_bd[b // 2][base: base + L, c * 128: (c + 1) * 128],
                        pt[base: base + L, :])

            # gi_emb = W_ihe.T @ embT (+ b_ih + [b_hh folded for r,z])
            for mc in range(MG):
                p = sps.tile([128, NT], F32, tag="p_gie")
                for kc in range(KD):
                    nc.tensor.matmul(
                        p[:],
                        wihe[:, kc * 3 * DEC + mc * 128: kc * 3 * DEC + mc * 128 + 128],
                        embT[:, kc * NT: (kc + 1) * NT],
                        start=(kc == 0), stop=(kc == KD - 1))
                nc.vector.tensor_scalar(
                    gi_emb[:, mc * NT: (mc + 1) * NT], p[:],
                    biasgi[:, mc: mc + 1], None, Op.add)

        # ------------------------------ recurrence ------------------------------
        nc.sync.dma_start(wfc_sb[:], d_wfc[:])
        with tc.tile_pool(name="rsb", bufs=3) as rsb, \
             tc.tile_pool(name="rps", bufs=1, space="PSUM") as rps:
            for t in range(n_steps if do_rec else 0):
                hprev = [hist[:, kc * hcol + 8 * t: kc * hcol + 8 * t + 8] for kc in range(KD)]

                # gh (r,z and n) and att2, weight-stationary bf16. The gi_emb
                # slice and b_hh_n enter psum via identity matmuls (frees the
                # DVE pre-add chain; ACT reads gates straight from psum).
                gie = gi_emb[:].rearrange("p (mc tb) -> p mc tb", mc=MG)
                p_ghrz = rps.tile([128, 64], F32, tag="p_ghrz", bufs=2)
                p_ghn = rps.tile([128, 32], F32, tag="p_ghn")
                p_att2 = rps.tile([128, MA * BL], F32, tag="p_att2")
                nc.tensor.matmul(
                    p_ghrz[:], ident_f[:],
                    gie[:, 0:8, 8 * t: 8 * t + 8],
                    start=True, stop=False)
                nc.tensor.matmul(
                    p_ghn[:], ident_f[:],
                    bhhnbc[:],
                    start=True, stop=False)
                for mc in range(MA):
                    for kc in range(KD):
                        nc.tensor.matmul(
                            p_att2[:, mc * 8: mc * 8 + 8],
                            wda[:, (kc * MA + mc) * 128: (kc * MA + mc) * 128 + 128],
                            hprev[kc], start=(kc == 0), stop=(kc == KD - 1))
                for mc in range(8):
                    for kc in range(KD):
                        nc.tensor.matmul(
                            p_ghrz[:, mc * 8: mc * 8 + 8],
                            whh[:, (kc * MG + mc) * 128: (kc * MG + mc) * 128 + 128],
                            hprev[kc], start=False, stop=False)
                for mc in range(8, MG):
                    for kc in range(KD):
                        nc.tensor.matmul(
                            p_ghn[:, (mc - 8) * 8: (mc - 8) * 8 + 8],
                            whh[:, (kc * MG + mc) * 128: (kc * MG + mc) * 128 + 128],
                            hprev[kc], start=False,
                            stop=(kc == KD - 1 and mc == MG - 1))

                # tanh(att1 + att2 broadcast over l); b_da+b_ea pre-folded into
                # att1. Split by att-chunk so the second TT overlaps the first
                # tanh (shorter critical path into the score matmuls).
                targ = rsb.tile([128, MA * NL], BF16, tag="targ")
                for c in range(MA):
                    nc.vector.tensor_tensor(
                        targ[:, c * NL: (c + 1) * NL].rearrange(
                            "p (b l) -> p b l", b=BL, l=L),
                        att1[:, c * NL: (c + 1) * NL].rearrange(
                            "p (b l) -> p b l", b=BL, l=L),
                        p_att2[:, c * BL: (c + 1) * BL].unsqueeze(2)
                        .broadcast_to([128, BL, L]),
                        Op.add)
                    nc.scalar.activation(
                        tanh_sb[:, c * 512: (c + 1) * 512].rearrange(
                            "p (b l) -> p b l", b=BL, l=64)[:, :, 0:L],
                        targ[:, c * NL: (c + 1) * NL].rearrange(
                            "p (b l) -> p b l", b=BL, l=L),
                        AF.Tanh)

                # scores -> psum [128 rows, pair cols]: one MM per (pair, kc);
                # lhsT is the 64-stride padded pair block (odd b -> rows 64+)
                p_sc = rps.tile([128, NPAIR], F32, tag="p_sc")
                for j in range(NPAIR):
                    for kc in range(MA):
                        nc.tensor.matmul(
                            p_sc[:, j: j + 1],
                            tanh_sb[:, kc * 512 + j * 128: kc * 512 + j * 128 + 128],
                            wfa[:, kc: kc + 1],
                            start=(kc == 0), stop=(kc == MA - 1))

                # exp (no max-subtraction: scores are tiny); block-diag layout kept zero
                nc.scalar.activation(expe[0:L, 0:BL:2], p_sc[0:L, 0:NPAIR], AF.Exp)
                nc.scalar.activation(expe[64:64 + L, 1:BL:2], p_sc[64:64 + L, 0:NPAIR], AF.Exp)

                # denominator broadcast to all partitions in one matmul
                # (lhsT = all-ones [128,128]), then reciprocal psum->sbuf
                p_small = rps.tile([128, BL], F32, tag="p_small")
                nc.tensor.matmul(p_small[:], ones_mat_bf[:], expe[:], start=True, stop=True)
                rb_sb = rsb.tile([128, BL], F32, tag="rb_sb")
                nc.vector.reciprocal(rb_sb[:], p_small[:])

                # context (block-diag pairs) and normalization
                p_ctx = rps.tile([128, KD * BL], F32, tag="p_ctx")
                for j in range(NPAIR):
                    for c in range(KD):
                        nc.tensor.matmul(
                            p_ctx[:, c * 8 + 2 * j: c * 8 + 2 * j + 2],
                            enc_bd[j][:, c * 128: (c + 1) * 128],
                            expe[:, 2 * j: 2 * j + 2],
                            start=True, stop=True)
                x_ctx = rsb.tile([128, KD * BL], BF16, tag="x_ctx")
                nc.vector.tensor_tensor(
                    x_ctx[:].rearrange("p (c b) -> p c b", c=KD),
                    p_ctx[:].rearrange("p (c b) -> p c b", c=KD),
                    rb_sb[:].unsqueeze(1).broadcast_to([128, KD, BL]),
                    Op.mult)

                # gi_ctx: r,z accumulate onto p_ghrz; n into p_gin (pre-loaded
                # with the gi_emb n-slice via identity matmul)
                p_gin = rps.tile([128, 32], F32, tag="p_gin")
                nc.tensor.matmul(
                    p_gin[:], ident_f[:],
                    gie[:, 8:MG, 8 * t: 8 * t + 8],
                    start=True, stop=False)
                for mc in range(8):
                    for kc in range(KD):
                        nc.tensor.matmul(
                            p_ghrz[:, mc * 8: mc * 8 + 8],
                            wihc[:, (kc * MG + mc) * 128: (kc * MG + mc) * 128 + 128],
                            x_ctx[:, kc * 8: kc * 8 + 8], start=False,
                            stop=(kc == KD - 1 and mc == 7))
                for mc in range(8, MG):
                    for kc in range(KD):
                        nc.tensor.matmul(
                            p_gin[:, (mc - 8) * 8: (mc - 8) * 8 + 8],
                            wihc[:, (kc * MG + mc) * 128: (kc * MG + mc) * 128 + 128],
                            x_ctx[:, kc * 8: kc * 8 + 8], start=False,
                            stop=(kc == KD - 1 and mc == MG - 1))

                # gates: t_rz = tanh(0.5 * rz_full) straight from psum
                t_rz = rsb.tile([128, 64], F32, tag="t_rz")
                nc.scalar.activation(t_rz[:], p_ghrz[:], AF.Tanh, scale=0.5)
                # r' and z' sigmoids in one affine op: 0.5*t + 0.5. The
                # n-gate chain (vv -> n_arg -> tanh) is the critical path, so
                # it issues on DVE before the off-chain zm/w1 ops, which then
                # execute under the ACT tanh.
                trz1 = rsb.tile([128, 64], F32, tag="trz1")
                nc.vector.tensor_scalar(trz1[:], t_rz[:], 0.5, 0.5, Op.mult, Op.add)
                vv = rsb.tile([128, 32], F32, tag="vv")
                nc.vector.tensor_tensor(vv[:], trz1[:, 0:32], p_ghn[:], Op.mult)
                n_arg = rsb.tile([128, 32], F32, tag="n_arg")
                nc.vector.tensor_tensor(n_arg[:], vv[:], p_gin[:], Op.add)
                n_g = rsb.tile([128, 32], F32, tag="n_g")
                nc.scalar.activation(n_g[:], n_arg[:], AF.Tanh)
                zm = rsb.tile([128, 32], F32, tag="zm")
                nc.vector.tensor_scalar(zm[:], t_rz[:, 32:64], -0.5, 0.5, Op.mult, Op.add)
                w1 = rsb.tile([128, 32], F32, tag="w1")
                nc.vector.tensor_tensor(
                    w1[:].rearrange("p (c b) -> p c b", c=KD),
                    hist[:].rearrange("p (c tb) -> p c tb", c=KD)
                    [:, :, 8 * t: 8 * t + 8],
                    trz1[:, 32:64].rearrange("p (c b) -> p c b", c=KD), Op.mult)
                # h_new tail uses zm/w1 computed under the ACT tanh
                # h_new = n*(1-z') + h*z' -> written straight into bf16 history
                u_g = rsb.tile([128, 32], F32, tag="u_g")
                nc.vector.tensor_tensor(u_g[:], n_g[:], zm[:], Op.mult)
                nc.vector.tensor_tensor(
                    hist[:].rearrange("p (c tb) -> p c tb", c=KD)
                    [:, :, 8 * (t + 1): 8 * (t + 1) + 8],
                    u_g[:].rearrange("p (c b) -> p c b", c=KD),
                    w1[:].rearrange("p (c b) -> p c b", c=KD), Op.add)

        # ------------------------------ fc phase ------------------------------
        nrem = V - (NCK - 1) * NV  # last tile width (10000 = 19*512 + 272)
        with tc.tile_pool(name="fsb", bufs=4) as fsb, \
             tc.tile_pool(name="fps", bufs=6, space="PSUM") as fps:
            n_mblk = (NT + 99) // 100
            for nck in range(NCK if do_fc else 0):
                nv = NV if nck < NCK - 1 else nrem
                for m in range(n_mblk):
                    mm = min(100, NT - m * 100)
                    p = fps.tile([128, NV], F32, tag="p_fc")
                    for kc in range(KD):
                        nc.tensor.matmul(
                            p[0:mm, 0:nv],
                            hist[:, kc * hcol + 8 + 100 * m: kc * hcol + 8 + 100 * m + mm],
                            wfc_sb[:, kc * V + nck * NV: kc * V + nck * NV + nv],
                            start=(kc == 0), stop=(kc == KD - 1))
                    lg = fsb.tile([128, NV], BF16, tag="lg")
                    nc.vector.tensor_tensor(
                        lg[0:mm, 0:nv], p[0:mm, 0:nv],
                        bfcb[0:mm, nck * NV: nck * NV + nv], Op.add)
                    nc.sync.dma_start(
                        d_logits[m * 100: m * 100 + mm, nck * NV: nck * NV + nv],
                        lg[0:mm, 0:nv])

    nc.finalize()
    return nc


# ------------------------------ host-side prep ------------------------------

def _chunk_lhs(w, k):
    """[K, M] -> [128, (K/128)*M] with col = kc*M + m."""
    K, M = w.shape
    return np.ascontiguousarray(w.reshape(k, 128, M).transpose(1, 0, 2).reshape(128, k * M))


def _chunk_lhs_sq(w, k, mchunks):
    """[K, M] -> [128, k*mchunks*128] with col = (kc*mchunks+mc)*128 + j."""
    K, M = w.shape
    return np.ascontiguousarray(
        w.reshape(k, 128, mchunks, 128).transpose(1, 0, 2, 3).reshape(128, k * mchunks * 128))


def _bf(x):
    return np.ascontiguousarray(x.astype(ml_dtypes.bfloat16))


def host_prep(inputs, n_steps=T):
    i = {k: np.asarray(v) for k, v in inputs.items()}
    sf = i["spatial_feats"].astype(np.float32)          # [64, 49, 2048]
    cap = i["captions"].astype(np.int64)                # [64, 50]
    W_feat, b_feat = i["W_feat"].astype(np.float32), i["b_feat"].astype(np.float32)
    W_ea, b_ea = i["W_ea"].astype(np.float32), i["b_ea"].astype(np.float32)
    W_da, b_da = i["W_da"].astype(np.float32), i["b_da"].astype(np.float32)
    W_fa = i["W_fa"].astype(np.float32)
    emb = i["emb"].astype(np.float32)
    W_ih, W_hh = i["W_ih"].astype(np.float32), i["W_hh"].astype(np.float32)
    b_ih, b_hh = i["b_ih"].astype(np.float32), i["b_hh"].astype(np.float32)
    W_fc, b_fc = i["W_fc"].astype(np.float32), i["b_fc"].astype(np.float32)

    shared = {
        "wfeat": _bf(_chunk_lhs(W_feat, KE)),
        "wea": _bf(_chunk_lhs(W_ea, KD)),
        "wihe": _bf(_chunk_lhs(np.ascontiguousarray(W_ih[:, :EMB].T), KD)),
        "wihc": _bf(_chunk_lhs_sq(np.ascontiguousarray(W_ih[:, EMB:].T), KD, MG)),
        "whh": _bf(_chunk_lhs_sq(np.ascontiguousarray(W_hh.T), KD, MG)),
        "wda": _bf(_chunk_lhs_sq(W_da, KD, MA)),
        "wfa": _bf(W_fa.reshape(MA, 128).T),
        "wfc": _bf(W_fc.reshape(KD, 128, V).transpose(1, 0, 2).reshape(128, KD * V)),
        "bfeat": np.ascontiguousarray(b_feat.reshape(KD, 128).T),
        "bea": np.ascontiguousarray((b_ea + b_da).reshape(MA, 128).T),
        "biasgi": np.ascontiguousarray(
            (b_ih + np.concatenate([b_hh[:2 * DEC], np.zeros(DEC, np.float32)])).reshape(MG, 128).T),
        "bhhnbc": _bf(
            np.repeat(b_hh[2 * DEC:].reshape(4, 128).T[:, :, None], BL, axis=2).reshape(128, 4 * BL)),
        "bfc": _bf(b_fc.reshape(1, V)),
    }
    in_maps = []
    for c in range(NCORES):
        sl = slice(c * BL, (c + 1) * BL)
        sfT = sf[sl].reshape(NL, ENC).T                      # [2048, 392]
        embs = emb[cap[sl][:, :n_steps]]                     # [8, n_steps, 512]
        embT = embs.transpose(1, 0, 2).reshape(BL * n_steps, EMB).T   # [512, NT]
        m = dict(shared)
        m["spatialT"] = _bf(sfT.reshape(KE, 128, NL).transpose(1, 0, 2).reshape(128, KE * NL))
        m["embT"] = _bf(embT.reshape(KD, 128, BL * n_steps).transpose(1, 0, 2).reshape(128, KD * BL * n_steps))
        in_maps.append(m)
    return in_maps


_PROG_CACHE = {}


def _get_prog(n_steps=T):
    if n_steps not in _PROG_CACHE:
        _PROG_CACHE[n_steps] = build_program(n_steps)
    return _PROG_CACHE[n_steps]


def kernel(**inputs):
    from concourse.bass_utils import run_bass_kernel_spmd
    nc = _get_prog(T)
    in_maps = host_prep(inputs, T)
    try:
        res = run_bass_kernel_spmd(nc, in_maps, core_ids=list(range(NCORES)))
    except Exception:
        # transient device errors (e.g. NRT_EXEC_UNIT_UNRECOVERABLE from a
        # previously wedged core) usually clear on retry
        res = run_bass_kernel_spmd(nc, in_maps, core_ids=list(range(NCORES)))
    outs = []
    for c in range(NCORES):
        lg = res.results[c]["logits"]                       # [400, 10000], row = 8t+b
        outs.append(lg.reshape(T, BL, V).transpose(1, 0, 2))  # [8, 50, 10000]
    return np.concatenate(outs, axis=0).astype(np.float32)    # [64, 50, 10000]



# revision 24
# speedup vs baseline: 1.2527x; 1.2527x over previous
"""Trainium2 Bass kernel for nn_CaptionModel (GRU + Bahdanau attention caption decoder).

Sharding: pure data-parallel over batch. B=64 -> 8 cores x 8 rows each; no
collectives (50 sequential steps cannot afford the ~5us/call collective floor).

Per-core plan (feature-major: features on partitions, local batch b=8 on free):
  setup:  enc = W_feat.T @ spatialT (+b_feat)        [512, 392]
          att1 = W_ea.T @ enc (+b_ea)                [256, 392] bf16
          enc_b[b]: per-batch [l, d] tiles (rows 0:49) for the context matmuls
          gi_emb = W_ih[:, :EMB].T @ embT (+biases)  [1536, 400] f32
  50 steps (weight-stationary matmuls, bf16 weights):
          att2 = W_da.T @ h (8 mm, issued first: longest chain)
          gh   = W_hh.T @ h (48 mm; n-block pre-scaled 0.5 for the fused gate)
          fc (lagged LAG steps): logitsT[vocab-part, b] = wfc.T @ h  (316 mm
              of N=8 in the PE idle window while ACT runs the attention tanh)
          tanh(att1 + att2) -> scores = tanhT @ W_fa  (per-b mm into [l, b] psum)
          one exp -> denom (ones mm) + context (per-b mm) -> recip -> x_ctx
          gi_ctx = W_ihc.T @ x_ctx (48 mm, accumulated with gh in psum for r,z)
          gates: sigmoid via 0.5+0.5*tanh(x/2); vv fused via scalar_tensor_tensor
          h stored fp32-free; bf16 copy appended to H_hist
  tail:   fc for the last LAG steps.

The second attention add runs on the (otherwise idle) GPSIMD/Pool engine from
an SBUF copy of att2 (Pool cannot read PSUM). fc psum is staged to SBUF by one
DVE and one ACT copy per step, then DMA'd out as [vocab-chunk, t, b]-major
bf16; the host de-transposes, slices vocab padding, and adds b_fc.

kernel() accepts FULL inputs, does host-side layout prep/sharding (incl. the
embedding-table gather), runs the same NEFF SPMD on cores 0-7, unshards.
"""

import contextlib

import ml_dtypes
import numpy as np

import concourse.bass as bass
import concourse.mybir as mybir
from concourse import bacc
from concourse.alu_op_type import AluOpType as Op
from concourse.masks import make_identity
from concourse.tile import TileContext

AF = mybir.ActivationFunctionType
F32 = mybir.dt.float32
BF16 = mybir.dt.bfloat16

B, L, ENC, DEC, EMB, ATT, V, T = 64, 49, 2048, 512, 512, 256, 10000, 50
NCORES = 8
BL = B // NCORES          # 8 local batch rows
NL = BL * L               # 392
KE = ENC // 128           # 16 K-chunks for enc matmul
KD = DEC // 128           # 4 K-chunks over DEC
MG = (3 * DEC) // 128     # 12 M-chunks over gates
MA = ATT // 128           # 2 M-chunks over ATT
HCOL = 8 * (T + 1)        # 408 cols per chunk in H history
VB = (V + 127) // 128     # 79 vocab 128-chunks (last is 16 wide)
VBA = 40                  # vocab chunks in fc psum tile A (tile B gets 39)
FCW = VB * BL             # 632 logit cols per step (vb-major, b-minor)
LAG = 4                   # fc trails the recurrence by LAG steps


def build_program(n_steps=T, dump_hist=False, dump_attn=False):
    nc = bacc.Bacc()
    NT = BL * n_steps      # t*8+b columns
    hcol = 8 * (n_steps + 1)

    # ---------------- DRAM I/O (per-core, host-prepped layouts) ----------------
    d_spatialT = nc.dram_tensor("spatialT", [128, KE * NL], BF16, kind="ExternalInput")
    d_embT = nc.dram_tensor("embT", [128, KD * NT], BF16, kind="ExternalInput")
    d_wfeat = nc.dram_tensor("wfeat", [128, KE * DEC], BF16, kind="ExternalInput")
    d_wea = nc.dram_tensor("wea", [128, KD * ATT], BF16, kind="ExternalInput")
    d_wihe = nc.dram_tensor("wihe", [128, KD * 3 * DEC], BF16, kind="ExternalInput")
    d_wihc = nc.dram_tensor("wihc", [128, KD * MG * 128], BF16, kind="ExternalInput")
    d_whh = nc.dram_tensor("whh", [128, KD * MG * 128], BF16, kind="ExternalInput")
    d_wda = nc.dram_tensor("wda", [128, KD * MA * 128], BF16, kind="ExternalInput")
    d_wfa = nc.dram_tensor("wfa", [128, MA], BF16, kind="ExternalInput")
    d_wfc = nc.dram_tensor("wfc", [128, KD * V], BF16, kind="ExternalInput")
    d_bfeat = nc.dram_tensor("bfeat", [128, KD], F32, kind="ExternalInput")
    d_bea = nc.dram_tensor("bea", [128, MA], F32, kind="ExternalInput")
    d_biasgi = nc.dram_tensor("biasgi", [128, MG], F32, kind="ExternalInput")
    d_bhhnbc = nc.dram_tensor("bhhnbc", [128, 4 * BL], BF16, kind="ExternalInput")
    d_logits = nc.dram_tensor("logits", [128, n_steps * FCW], BF16, kind="ExternalOutput")
    d_histd = nc.dram_tensor("histd", [128, 32 * (n_steps + 1)], BF16,
                             kind="ExternalOutput") if dump_hist else None
    if dump_attn:
        d_targd = nc.dram_tensor("targd", [128, MA * NL], BF16, kind="ExternalOutput")
        d_tanhd = nc.dram_tensor("tanhd", [128, MA * NL], BF16, kind="ExternalOutput")
        d_exped = nc.dram_tensor("exped", [128, BL], BF16, kind="ExternalOutput")
        d_xctxd = nc.dram_tensor("xctxd", [128, KD * BL], BF16, kind="ExternalOutput")
        d_att1d = nc.dram_tensor("att1d", [128, MA * NL], BF16, kind="ExternalOutput")

    with TileContext(nc) as tc, contextlib.ExitStack() as ctx:
        const = ctx.enter_context(tc.tile_pool(name="const", bufs=1))
        state = ctx.enter_context(tc.tile_pool(name="state", bufs=1))

        # persistent weights / constants in SBUF
        wihc = const.tile([128, KD * MG * 128], BF16, tag="wihc")
        whh = const.tile([128, KD * MG * 128], BF16, tag="whh")
        wda = const.tile([128, KD * MA * 128], BF16, tag="wda")
        wfa = const.tile([128, MA], BF16, tag="wfa")
        bhhnbc = const.tile([128, 4 * BL], BF16, tag="bhhnbc")
        biasgi = const.tile([128, MG], F32, tag="biasgi")
        ident_f = const.tile([128, 128], BF16, tag="ident_f")
        ones_mat_bf = const.tile([128, 128], BF16, tag="ones_mat")
        wfc_sb = const.tile([128, KD * V], BF16, tag="wfc_sb")

        # persistent activations / state
        att1 = state.tile([128, MA * NL], BF16, tag="att1")
        enc_b = [state.tile([128, DEC], BF16, tag=f"encb{b}", name=f"encb{b}")
                 for b in range(BL)]
        gi_emb = state.tile([128, MG * NT], BF16, tag="gi_emb")
        hist = state.tile([128, 32 * (n_steps + 1)], BF16, tag="hist")
        expe = state.tile([128, BL], BF16, tag="expe")
        targ = state.tile([128, MA * NL], BF16, tag="targ")
        tanh_sb = state.tile([128, MA * NL], BF16, tag="tanh_sb")

        make_identity(nc, ident_f[:])
        nc.gpsimd.memset(ones_mat_bf[:], 1.0)
        nc.gpsimd.memset(hist[:], 0.0)

        # ------------------------------ setup phase ------------------------------
        # DMA order = DMA-engine service order: setup-compute inputs first,
        # recurrence weights next, the big fc weight last (hidden by LAG).
        with tc.tile_pool(name="ssb", bufs=1) as ssb:
            spatialT = ssb.tile([128, KE * NL], BF16, tag="spatialT")
            embT = ssb.tile([128, KD * NT], BF16, tag="embT")
            wfeat = ssb.tile([128, KE * DEC], BF16, tag="wfeat")
            wea = ssb.tile([128, KD * ATT], BF16, tag="wea")
            wihe = ssb.tile([128, KD * 3 * DEC], BF16, tag="wihe")
            bfeat = ssb.tile([128, KD], F32, tag="bfeat")
            bea = ssb.tile([128, MA], F32, tag="bea")
            for dst, src in [(biasgi, d_biasgi), (bfeat, d_bfeat), (bea, d_bea),
                             (wfa, d_wfa), (bhhnbc, d_bhhnbc),
                             (embT, d_embT), (wihe, d_wihe),
                             (spatialT, d_spatialT), (wfeat, d_wfeat),
                             (wea, d_wea),
                             (wda, d_wda), (whh, d_whh), (wihc, d_wihc)]:
                nc.sync.dma_start(dst[:], src[:])
            nc.sync.dma_start(wfc_sb[:], d_wfc[:])

            enc_fm = ssb.tile([128, KD * NL], BF16, tag="enc_fm")
            sps = ctx_sps = tc.tile_pool(name="sps", bufs=1, space="PSUM")
            sps = sps.__enter__()
            # gi_emb = W_ihe.T @ embT (+ b_ih + [b_hh folded for r,z]) —
            # first: embT/wihe are the first big DMAs to land
            for mc in range(MG):
                p = sps.tile([128, NT], F32, tag="p_gie", bufs=2)
                for kc in range(KD):
                    nc.tensor.matmul(
                        p[:],
                        wihe[:, kc * 3 * DEC + mc * 128: kc * 3 * DEC + mc * 128 + 128],
                        embT[:, kc * NT: (kc + 1) * NT],
                        start=(kc == 0), stop=(kc == KD - 1))
                if mc % 2 == 0:
                    nc.vector.tensor_scalar(
                        gi_emb[:, mc * NT: (mc + 1) * NT], p[:],
                        biasgi[:, mc: mc + 1], None, Op.add)
                else:
                    nc.scalar.activation(
                        gi_emb[:, mc * NT: (mc + 1) * NT], p[:],
                        AF.Identity, bias=biasgi[:, mc: mc + 1])

            # enc = W_feat.T @ spatialT (+ b_feat), feature-major
            p_encs = [sps.tile([128, NL], F32, tag=f"p_enc{mc}", name=f"p_enc{mc}", bufs=1)
                      for mc in range(KD)]
            for kc in range(KE):
                for mc in range(KD):
                    nc.tensor.matmul(
                        p_encs[mc][:],
                        wfeat[:, kc * DEC + mc * 128: kc * DEC + mc * 128 + 128],
                        spatialT[:, kc * NL: (kc + 1) * NL],
                        start=(kc == 0), stop=(kc == KE - 1))
            for mc in range(KD):
                if mc % 2 == 0:
                    nc.vector.tensor_scalar(
                        enc_fm[:, mc * NL: (mc + 1) * NL], p_encs[mc][:],
                        bfeat[:, mc: mc + 1], None, Op.add)
                else:
                    nc.scalar.activation(
                        enc_fm[:, mc * NL: (mc + 1) * NL], p_encs[mc][:],
                        AF.Identity, bias=bfeat[:, mc: mc + 1])

            # att1 = W_ea.T @ enc (+ b_ea + b_da)  -> bf16 [att-chunk, b*49+l]
            for mc in range(MA):
                p = sps.tile([128, NL], F32, tag="p_att1", bufs=2)
                for kc in range(KD):
                    nc.tensor.matmul(
                        p[:],
                        wea[:, kc * ATT + mc * 128: kc * ATT + mc * 128 + 128],
                        enc_fm[:, kc * NL: (kc + 1) * NL],
                        start=(kc == 0), stop=(kc == KD - 1))
                if mc % 2 == 0:
                    nc.vector.tensor_scalar(
                        att1[:, mc * NL: (mc + 1) * NL], p[:],
                        bea[:, mc: mc + 1], None, Op.add)
                else:
                    nc.scalar.activation(
                        att1[:, mc * NL: (mc + 1) * NL], p[:],
                        AF.Identity, bias=bea[:, mc: mc + 1])

            ctx_sps.__exit__(None, None, None)
            sps2 = ctx_sps2 = tc.tile_pool(name="sps2", bufs=1, space="PSUM")
            sps2 = sps2.__enter__()
            # enc_b[b]: rows 0:49 = enc[b] as [l, d]: 4 transpose matmuls into
            # one psum bank per b, then a single wide copy (DVE/ACT alternating)
            for b in range(BL):
                pt = sps2.tile([128, DEC], F32, tag="p_tr", bufs=2)
                for c in range(KD):
                    nc.tensor.matmul(
                        pt[0:L, c * 128: (c + 1) * 128],
                        enc_fm[:, c * NL + b * L: c * NL + b * L + L],
                        ident_f[:], start=True, stop=True)
                if b % 2 == 0:
                    nc.vector.tensor_copy(enc_b[b][0:L, :], pt[0:L, :])
                else:
                    nc.scalar.activation(enc_b[b][0:L, :], pt[0:L, :], AF.Identity)


            ctx_sps2.__exit__(None, None, None)

        # ------------------------------ recurrence ------------------------------
        def fc_mms(s, fps):
            """logitsT for timestep s from hist col block s+1; vb-major psum."""
            p_fcs = [fps.tile([128, VBA * BL], F32, tag="p_fcA", name="p_fcA"),
                     fps.tile([128, (VB - VBA) * BL], F32, tag="p_fcB", name="p_fcB")]
            for vb in range(VB):
                half = 0 if vb < VBA else 1
                col = (vb - (0 if vb < VBA else VBA)) * BL
                nv = 128 if vb < VB - 1 else V - (VB - 1) * 128
                for kc in range(KD):
                    nc.tensor.matmul(
                        p_fcs[half][0:nv, col: col + BL],
                        wfc_sb[:, kc * V + vb * 128: kc * V + vb * 128 + nv],
                        hist[:, 32 * (s + 1) + 8 * kc: 32 * (s + 1) + 8 * kc + 8],
                        start=(kc == 0), stop=(kc == KD - 1))
            return p_fcs

        _NOSYNC = mybir.DependencyInfo(sync=False, no_sync=True)

        def fc_out(s, p_fcs, rsb, after_dve=None, after_act=None):
            fcsb = rsb.tile([128, FCW], BF16, tag="fcsb", bufs=3)
            cpA = nc.vector.tensor_copy(fcsb[:, 0: VBA * BL], p_fcs[0][:])
            cpB = nc.scalar.copy(fcsb[:, VBA * BL: FCW], p_fcs[1][:])
            # keep the staging copies out of the attention chain: same-engine
            # ordering edges (no runtime sems) behind the 2nd add / the exp
            if after_dve is not None:
                cpA.ins.add_dependency(after_dve.ins.name, _NOSYNC)
            if after_act is not None:
                cpB.ins.add_dependency(after_act.ins.name, _NOSYNC)
            nc.sync.dma_start(d_logits[:, s * FCW: (s + 1) * FCW], fcsb[:])

        with tc.tile_pool(name="rsb", bufs=3) as rsb, \
             tc.tile_pool(name="rps", bufs=1, space="PSUM") as rps, \
             tc.tile_pool(name="fps", bufs=2, space="PSUM") as fps:
            for t in range(n_steps):
                hprev = [hist[:, 32 * t + 8 * kc: 32 * t + 8 * kc + 8] for kc in range(KD)]
                gie = gi_emb[:].rearrange("p (mc tb) -> p mc tb", mc=MG)

                # fc for the lagged step first: its matmuls have no
                # dependencies on this step, so they execute in the PE idle
                # window while the previous step's gate chain runs on DVE/ACT.
                # (the psum->SBUF copies are issued after the attention adds
                # so they don't interleave into the DVE/ACT chain)
                p_fcs = fc_mms(t - LAG, fps) if t >= LAG else None

                # One packed psum bank for the small per-step accumulators:
                # att2 0:16 | sc 16:24 | small 24:32 | ctx 32:64 | gin 64:96 | ghn 96:128
                p_mix = rps.tile([128, 128], F32, tag="p_mix")

                for mc in range(MA):
                    for kc in range(KD):
                        nc.tensor.matmul(
                            p_mix[:, mc * 8: mc * 8 + 8],
                            wda[:, (kc * MA + mc) * 128: (kc * MA + mc) * 128 + 128],
                            hprev[kc], start=(kc == 0), stop=(kc == KD - 1))

                # gh (r,z and n): gi_emb slice and 0.5*b_hh_n enter psum via
                # identity matmuls; ACT reads gates straight from psum.
                p_ghrz = rps.tile([128, 64], F32, tag="p_ghrz", bufs=2)
                nc.tensor.matmul(
                    p_ghrz[:], ident_f[:], gie[:, 0:8, 8 * t: 8 * t + 8],
                    start=True, stop=False)
                nc.tensor.matmul(
                    p_mix[:, 96:128], ident_f[:], bhhnbc[:], start=True, stop=False)
                for mc in range(8):
                    for kc in range(KD):
                        nc.tensor.matmul(
                            p_ghrz[:, mc * 8: mc * 8 + 8],
                            whh[:, (kc * MG + mc) * 128: (kc * MG + mc) * 128 + 128],
                            hprev[kc], start=False, stop=False)
                for mc in range(8, MG):
                    for kc in range(KD):
                        nc.tensor.matmul(
                            p_mix[:, 96 + (mc - 8) * 8: 96 + (mc - 8) * 8 + 8],
                            whh[:, (kc * MG + mc) * 128: (kc * MG + mc) * 128 + 128],
                            hprev[kc], start=False,
                            stop=(kc == KD - 1 and mc == MG - 1))

                # att2 bf16 copy for Pool (Pool cannot read PSUM); chunk-0 add
                # on DVE straight from psum, chunk-1 add on Pool, so the two
                # adds run concurrently and tanh c1 is not gated on DVE.
                for c in range(MA):
                    i_add = nc.vector.tensor_tensor(
                        targ[:, c * NL: (c + 1) * NL].rearrange(
                            "p (b l) -> p b l", b=BL, l=L),
                        att1[:, c * NL: (c + 1) * NL].rearrange(
                            "p (b l) -> p b l", b=BL, l=L),
                        p_mix[:, c * BL: (c + 1) * BL].unsqueeze(2)
                        .broadcast_to([128, BL, L]),
                        Op.add)
                    nc.scalar.activation(
                        tanh_sb[:, c * NL: (c + 1) * NL],
                        targ[:, c * NL: (c + 1) * NL], AF.Tanh)

                # scores -> psum [l rows 0:49, b cols]. b-outer: each column's
                # accumulation group is consecutive — PSUM groups are bank-
                # scoped, so interleaved open groups in one bank lose updates.
                prev_sc = None
                for b in range(BL):
                    for kc in range(MA):
                        i_sc = nc.tensor.matmul(
                            p_mix[0:L, 16 + b: 16 + b + 1],
                            tanh_sb[:, kc * NL + b * L: kc * NL + b * L + L],
                            wfa[:, kc: kc + 1],
                            start=(kc == 0), stop=(kc == MA - 1))
                        if prev_sc is not None:
                            i_sc.ins.add_dependency(prev_sc.ins.name, _NOSYNC)
                        prev_sc = i_sc

                # single exp (no max-subtraction: scores are tiny)
                i_exp = nc.scalar.activation(expe[0:L, 0:BL], p_mix[0:L, 16:24], AF.Exp)
                if p_fcs is not None:
                    fc_out(t - LAG, p_fcs, rsb, after_dve=i_add, after_act=i_exp)

                # denominator broadcast to all partitions in one matmul
                # (lhsT = ones [49, 128]); context matmuls run concurrently
                nc.tensor.matmul(p_mix[:, 24:32], ones_mat_bf[0:L, :], expe[0:L, :],
                                 start=True, stop=True)
                for b in range(BL):
                    for c in range(KD):
                        nc.tensor.matmul(
                            p_mix[:, 32 + c * 8 + b: 32 + c * 8 + b + 1],
                            enc_b[b][0:L, c * 128: (c + 1) * 128],
                            expe[0:L, b: b + 1],
                            start=True, stop=True)
                rb_sb = rsb.tile([128, BL], F32, tag="rb_sb")
                nc.vector.reciprocal(rb_sb[:], p_mix[:, 24:32])
                x_ctx = rsb.tile([128, KD * BL], BF16, tag="x_ctx")
                nc.vector.tensor_tensor(
                    x_ctx[:].rearrange("p (c b) -> p c b", c=KD),
                    p_mix[:, 32:64].rearrange("p (c b) -> p c b", c=KD),
                    rb_sb[:].unsqueeze(1).broadcast_to([128, KD, BL]),
                    Op.mult)

                # gi_ctx: r,z accumulate onto p_ghrz; n into p_gin (pre-loaded
                # with the gi_emb n-slice via identity matmul)
                nc.tensor.matmul(
                    p_mix[:, 64:96], ident_f[:], gie[:, 8:MG, 8 * t: 8 * t + 8],
                    start=True, stop=False)
                for mc in range(8):
                    for kc in range(KD):
                        nc.tensor.matmul(
                            p_ghrz[:, mc * 8: mc * 8 + 8],
                            wihc[:, (kc * MG + mc) * 128: (kc * MG + mc) * 128 + 128],
                            x_ctx[:, kc * 8: kc * 8 + 8], start=False,
                            stop=(kc == KD - 1 and mc == 7))
                for mc in range(8, MG):
                    for kc in range(KD):
                        nc.tensor.matmul(
                            p_mix[:, 64 + (mc - 8) * 8: 64 + (mc - 8) * 8 + 8],
                            wihc[:, (kc * MG + mc) * 128: (kc * MG + mc) * 128 + 128],
                            x_ctx[:, kc * 8: kc * 8 + 8], start=False,
                            stop=(kc == KD - 1 and mc == MG - 1))

                # gates: t_rz = tanh(0.5 * rz_full) straight from psum;
                # sigmoids are 0.5*t+0.5. The n-chain is
                #   vv = r'*gh_n = (t_r + 1) * (0.5*gh_n)   [one fused STT;
                #        the 0.5 is folded into whh's n-block host-side]
                #   n = tanh(vv + gi_n)
                # and issues on DVE before the off-chain z ops, which then
                # execute under the ACT tanh.
                t_rz = rsb.tile([128, 64], F32, tag="t_rz")
                nc.scalar.activation(t_rz[:], p_ghrz[:], AF.Tanh, scale=0.5)
                vv = rsb.tile([128, 32], F32, tag="vv")
                nc.vector.scalar_tensor_tensor(
                    vv[:], t_rz[:, 0:32], 1.0, p_mix[:, 96:128], Op.add, Op.mult)
                n_arg = rsb.tile([128, 32], F32, tag="n_arg")
                nc.vector.tensor_tensor(n_arg[:], vv[:], p_mix[:, 64:96], Op.add)
                n_g = rsb.tile([128, 32], F32, tag="n_g")
                nc.scalar.activation(n_g[:], n_arg[:], AF.Tanh)
                zm = rsb.tile([128, 32], F32, tag="zm")
                nc.vector.tensor_scalar(zm[:], t_rz[:, 32:64], -0.5, 0.5, Op.mult, Op.add)
                trz1_z = rsb.tile([128, 32], F32, tag="trz1_z")
                nc.vector.tensor_scalar(trz1_z[:], t_rz[:, 32:64], 0.5, 0.5, Op.mult, Op.add)
                w1 = rsb.tile([128, 32], F32, tag="w1")
                nc.vector.tensor_tensor(
                    w1[:], hist[:, 32 * t: 32 * t + 32], trz1_z[:], Op.mult)
                # h_new = n*(1-z') + h*z' -> written straight into bf16 history
                u_g = rsb.tile([128, 32], F32, tag="u_g")
                nc.vector.tensor_tensor(u_g[:], n_g[:], zm[:], Op.mult)
                nc.vector.tensor_tensor(
                    hist[:, 32 * (t + 1): 32 * (t + 1) + 32], u_g[:], w1[:], Op.add)

            # fc tail for the last LAG steps
            for s in range(max(0, n_steps - LAG), n_steps):
                fc_out(s, fc_mms(s, fps), rsb)
            if dump_hist:
                nc.sync.dma_start(d_histd[:], hist[:])
            if dump_attn:
                nc.sync.dma_start(d_targd[:], targ[:])
                nc.sync.dma_start(d_tanhd[:], tanh_sb[:])
                nc.sync.dma_start(d_exped[:], expe[:])
                nc.sync.dma_start(d_xctxd[:], x_ctx[:])
                nc.sync.dma_start(d_att1d[:], att1[:])

    nc.finalize()
    return nc


# ------------------------------ host-side prep ------------------------------

def _chunk_lhs(w, k):
    """[K, M] -> [128, (K/128)*M] with col = kc*M + m."""
    K, M = w.shape
    return np.ascontiguousarray(w.reshape(k, 128, M).transpose(1, 0, 2).reshape(128, k * M))


def _chunk_lhs_sq(w, k, mchunks):
    """[K, M] -> [128, k*mchunks*128] with col = (kc*mchunks+mc)*128 + j."""
    K, M = w.shape
    return np.ascontiguousarray(
        w.reshape(k, 128, mchunks, 128).transpose(1, 0, 2, 3).reshape(128, k * mchunks * 128))


def _bf(x):
    return np.ascontiguousarray(x.astype(ml_dtypes.bfloat16))


def host_prep(inputs, n_steps=T):
    i = {k: np.asarray(v) for k, v in inputs.items()}
    sf = i["spatial_feats"].astype(np.float32)          # [64, 49, 2048]
    cap = i["captions"].astype(np.int64)                # [64, 50]
    W_feat, b_feat = i["W_feat"].astype(np.float32), i["b_feat"].astype(np.float32)
    W_ea, b_ea = i["W_ea"].astype(np.float32), i["b_ea"].astype(np.float32)
    W_da, b_da = i["W_da"].astype(np.float32), i["b_da"].astype(np.float32)
    W_fa = i["W_fa"].astype(np.float32)
    emb = i["emb"].astype(np.float32)
    W_ih, W_hh = i["W_ih"].astype(np.float32), i["W_hh"].astype(np.float32)
    b_ih, b_hh = i["b_ih"].astype(np.float32), i["b_hh"].astype(np.float32)
    W_fc = i["W_fc"].astype(np.float32)

    # scale the n-block of W_hh / b_hh_n by 0.5: the fused gate op computes
    # r'*gh_n as (tanh+1)*(0.5*gh_n)
    W_hhT = np.ascontiguousarray(W_hh.T).copy()
    W_hhT[:, 2 * DEC:] *= 0.5

    shared = {
        "wfeat": _bf(_chunk_lhs(W_feat, KE)),
        "wea": _bf(_chunk_lhs(W_ea, KD)),
        "wihe": _bf(_chunk_lhs(np.ascontiguousarray(W_ih[:, :EMB].T), KD)),
        "wihc": _bf(_chunk_lhs_sq(np.ascontiguousarray(W_ih[:, EMB:].T), KD, MG)),
        "whh": _bf(_chunk_lhs_sq(W_hhT, KD, MG)),
        "wda": _bf(_chunk_lhs_sq(W_da, KD, MA)),
        "wfa": _bf(W_fa.reshape(MA, 128).T),
        "wfc": _bf(W_fc.reshape(KD, 128, V).transpose(1, 0, 2).reshape(128, KD * V)),
        "bfeat": np.ascontiguousarray(b_feat.reshape(KD, 128).T),
        "bea": np.ascontiguousarray((b_ea + b_da).reshape(MA, 128).T),
        "biasgi": np.ascontiguousarray(
            (b_ih + np.concatenate([b_hh[:2 * DEC], np.zeros(DEC, np.float32)])).reshape(MG, 128).T),
        "bhhnbc": _bf(
            np.repeat((0.5 * b_hh[2 * DEC:]).reshape(4, 128).T[:, :, None], BL, axis=2).reshape(128, 4 * BL)),
    }
    in_maps = []
    for c in range(NCORES):
        sl = slice(c * BL, (c + 1) * BL)
        sfT = sf[sl].reshape(NL, ENC).T                      # [2048, 392]
        embs = emb[cap[sl][:, :n_steps]]                     # [8, n_steps, 512]
        embT = embs.transpose(1, 0, 2).reshape(BL * n_steps, EMB).T   # [512, NT]
        m = dict(shared)
        m["spatialT"] = _bf(sfT.reshape(KE, 128, NL).transpose(1, 0, 2).reshape(128, KE * NL))
        m["embT"] = _bf(embT.reshape(KD, 128, BL * n_steps).transpose(1, 0, 2).reshape(128, KD * BL * n_steps))
        in_maps.append(m)
    return in_maps


def unshard(results, b_fc, n_steps=T):
    outs = []
    for c in range(NCORES):
        lg = results[c]["logits"]                          # [128, T*FCW] bf16
        lg = np.asarray(lg).reshape(128, n_steps, VB, BL)
        lg = lg.transpose(3, 1, 2, 0).reshape(BL, n_steps, VB * 128)[:, :, :V]
        outs.append(lg)
    out = np.concatenate(outs, axis=0).astype(np.float32)    # [64, T, 10000]
    out += np.asarray(b_fc).astype(np.float32)[None, None, :]
    return out


_PROG_CACHE = {}


def _get_prog(n_steps=T):
    if n_steps not in _PROG_CACHE:
        _PROG_CACHE[n_steps] = build_program(n_steps)
    return _PROG_CACHE[n_steps]


def kernel(**inputs):
    from concourse.bass_utils import run_bass_kernel_spmd
    nc = _get_prog(T)
    in_maps = host_prep(inputs, T)
    try:
        res = run_bass_kernel_spmd(nc, in_maps, core_ids=list(range(NCORES)))
    except Exception:
        # transient device errors (e.g. NRT_EXEC_UNIT_UNRECOVERABLE from a
        # previously wedged core) usually clear on retry
        res = run_bass_kernel_spmd(nc, in_maps, core_ids=list(range(NCORES)))
    return unshard(res.results, inputs["b_fc"], T)


# revision 36
# speedup vs baseline: 1.2689x; 1.0130x over previous
"""Trainium2 Bass kernel for nn_CaptionModel (GRU + Bahdanau attention caption decoder).

Sharding: pure data-parallel over batch. B=64 -> 8 cores x 8 rows each; no
collectives (50 sequential steps cannot afford the ~5us/call collective floor).

Per-core plan (feature-major: features on partitions, local batch b=8 on free):
  setup:  enc = W_feat.T @ spatialT (+b_feat)        [512, 392]
          att1 = W_ea.T @ enc (+b_ea)                [256, 392] bf16
          enc_b[b]: per-batch [l, d] tiles (rows 0:49) for the context matmuls
          gi_emb = W_ih[:, :EMB].T @ embT (+biases)  [1536, 400] f32
  50 steps (weight-stationary matmuls, bf16 weights):
          att2 = W_da.T @ h (8 mm, issued first: longest chain)
          gh   = W_hh.T @ h (48 mm; n-block pre-scaled 0.5 for the fused gate)
          fc (lagged LAG steps): logitsT[vocab-part, b] = wfc.T @ h  (316 mm
              of N=8 in the PE idle window while ACT runs the attention tanh)
          tanh(att1 + att2) -> scores = tanhT @ W_fa  (per-b mm into [l, b] psum)
          one exp -> denom (ones mm) + context (per-b mm) -> recip -> x_ctx
          gi_ctx = W_ihc.T @ x_ctx (48 mm, accumulated with gh in psum for r,z)
          gates: sigmoid via 0.5+0.5*tanh(x/2); vv fused via scalar_tensor_tensor
          h stored fp32-free; bf16 copy appended to H_hist
  tail:   fc for the last LAG steps.

The second attention add runs on the (otherwise idle) GPSIMD/Pool engine from
an SBUF copy of att2 (Pool cannot read PSUM). fc psum is staged to SBUF by one
DVE and one ACT copy per step, then DMA'd out as [vocab-chunk, t, b]-major
bf16; the host de-transposes, slices vocab padding, and adds b_fc.

kernel() accepts FULL inputs, does host-side layout prep/sharding (incl. the
embedding-table gather), runs the same NEFF SPMD on cores 0-7, unshards.
"""

import contextlib

import ml_dtypes
import numpy as np

import concourse.bass as bass
import concourse.mybir as mybir
from concourse import bacc
from concourse.alu_op_type import AluOpType as Op
from concourse.masks import make_identity
from concourse.tile import TileContext

AF = mybir.ActivationFunctionType
F32 = mybir.dt.float32
BF16 = mybir.dt.bfloat16

B, L, ENC, DEC, EMB, ATT, V, T = 64, 49, 2048, 512, 512, 256, 10000, 50
NCORES = 8
BL = B // NCORES          # 8 local batch rows
NL = BL * L               # 392
KE = ENC // 128           # 16 K-chunks for enc matmul
KD = DEC // 128           # 4 K-chunks over DEC
MG = (3 * DEC) // 128     # 12 M-chunks over gates
MA = ATT // 128           # 2 M-chunks over ATT
HCOL = 8 * (T + 1)        # 408 cols per chunk in H history
VB = (V + 127) // 128     # 79 vocab 128-chunks (last is 16 wide)
VBA = 40                  # vocab chunks in fc psum tile A (tile B gets 39)
FCW = VB * BL             # 632 logit cols per step (vb-major, b-minor)
LAG = 5                   # fc trails the recurrence by LAG steps


def build_program(n_steps=T, dump_hist=False, dump_attn=False):
    nc = bacc.Bacc()
    NT = BL * n_steps      # t*8+b columns
    hcol = 8 * (n_steps + 1)

    # ---------------- DRAM I/O (per-core, host-prepped layouts) ----------------
    d_spatialT = nc.dram_tensor("spatialT", [128, KE * NL], BF16, kind="ExternalInput")
    d_embT = nc.dram_tensor("embT", [128, KD * NT], BF16, kind="ExternalInput")
    d_wfeat = nc.dram_tensor("wfeat", [128, KE * DEC], BF16, kind="ExternalInput")
    d_wea = nc.dram_tensor("wea", [128, KD * ATT], BF16, kind="ExternalInput")
    d_wihe = nc.dram_tensor("wihe", [128, KD * 3 * DEC], BF16, kind="ExternalInput")
    d_wihc = nc.dram_tensor("wihc", [128, KD * MG * 128], BF16, kind="ExternalInput")
    d_whh = nc.dram_tensor("whh", [128, KD * MG * 128], BF16, kind="ExternalInput")
    d_wda = nc.dram_tensor("wda", [128, KD * MA * 128], BF16, kind="ExternalInput")
    d_wfa = nc.dram_tensor("wfa", [128, MA], BF16, kind="ExternalInput")
    d_wfc = nc.dram_tensor("wfc", [128, KD * V], BF16, kind="ExternalInput")
    d_bfeat = nc.dram_tensor("bfeat", [128, KD], F32, kind="ExternalInput")
    d_bea = nc.dram_tensor("bea", [128, MA], F32, kind="ExternalInput")
    d_biasgi = nc.dram_tensor("biasgi", [128, MG], F32, kind="ExternalInput")
    d_bhhnbc = nc.dram_tensor("bhhnbc", [128, 4 * BL], BF16, kind="ExternalInput")
    d_logits = nc.dram_tensor("logits", [128, n_steps * FCW], BF16, kind="ExternalOutput")
    d_histd = nc.dram_tensor("histd", [128, 32 * (n_steps + 1)], BF16,
                             kind="ExternalOutput") if dump_hist else None
    if dump_attn:
        d_targd = nc.dram_tensor("targd", [128, MA * NL], BF16, kind="ExternalOutput")
        d_tanhd = nc.dram_tensor("tanhd", [128, MA * NL], BF16, kind="ExternalOutput")
        d_exped = nc.dram_tensor("exped", [128, BL], BF16, kind="ExternalOutput")
        d_xctxd = nc.dram_tensor("xctxd", [128, KD * BL], BF16, kind="ExternalOutput")
        d_att1d = nc.dram_tensor("att1d", [128, MA * NL], BF16, kind="ExternalOutput")

    with TileContext(nc) as tc, contextlib.ExitStack() as ctx:
        const = ctx.enter_context(tc.tile_pool(name="const", bufs=1))
        state = ctx.enter_context(tc.tile_pool(name="state", bufs=1))

        # persistent weights / constants in SBUF
        wihc = const.tile([128, KD * MG * 128], BF16, tag="wihc")
        whh = const.tile([128, KD * MG * 128], BF16, tag="whh")
        wda = const.tile([128, KD * MA * 128], BF16, tag="wda")
        wfa = const.tile([128, MA], BF16, tag="wfa")
        bhhnbc = const.tile([128, 4 * BL], BF16, tag="bhhnbc")
        biasgi = const.tile([128, MG], F32, tag="biasgi")
        ident_f = const.tile([128, 128], BF16, tag="ident_f")
        ones_mat_bf = const.tile([128, 128], BF16, tag="ones_mat")
        wfc_sb = const.tile([128, KD * V], BF16, tag="wfc_sb")

        # persistent activations / state
        att1 = state.tile([128, MA * NL], BF16, tag="att1")
        enc_b = [state.tile([128, DEC], BF16, tag=f"encb{b}", name=f"encb{b}")
                 for b in range(BL)]
        gi_emb = state.tile([128, MG * NT], BF16, tag="gi_emb")
        hist = state.tile([128, 32 * (n_steps + 1)], BF16, tag="hist")
        expe = state.tile([128, BL], BF16, tag="expe")
        targ = state.tile([128, MA * NL], BF16, tag="targ")
        tanh_sb = state.tile([128, MA * NL], BF16, tag="tanh_sb")

        make_identity(nc, ident_f[:])
        nc.gpsimd.memset(ones_mat_bf[:], 1.0)
        nc.gpsimd.memset(hist[:], 0.0)

        # ------------------------------ setup phase ------------------------------
        # DMA order = DMA-engine service order: setup-compute inputs first,
        # recurrence weights next, the big fc weight last (hidden by LAG).
        with tc.tile_pool(name="ssb", bufs=1) as ssb:
            spatialT = ssb.tile([128, KE * NL], BF16, tag="spatialT")
            embT = ssb.tile([128, KD * NT], BF16, tag="embT")
            wfeat = ssb.tile([128, KE * DEC], BF16, tag="wfeat")
            wea = ssb.tile([128, KD * ATT], BF16, tag="wea")
            wihe = ssb.tile([128, KD * 3 * DEC], BF16, tag="wihe")
            bfeat = ssb.tile([128, KD], F32, tag="bfeat")
            bea = ssb.tile([128, MA], F32, tag="bea")
            for dst, src in [(biasgi, d_biasgi),
                             (embT, d_embT), (wihe, d_wihe),
                             (spatialT, d_spatialT), (wfeat, d_wfeat),
                             (bfeat, d_bfeat), (bea, d_bea),
                             (wfa, d_wfa), (bhhnbc, d_bhhnbc),
                             (wea, d_wea),
                             (wda, d_wda), (whh, d_whh), (wihc, d_wihc)]:
                nc.sync.dma_start(dst[:], src[:])
            nc.sync.dma_start(wfc_sb[:], d_wfc[:])

            enc_fm = ssb.tile([128, KD * NL], BF16, tag="enc_fm")
            sps = ctx_sps = tc.tile_pool(name="sps", bufs=1, space="PSUM")
            sps = sps.__enter__()
            # gi_emb = W_ihe.T @ embT (+ b_ih + [b_hh folded for r,z]) —
            # first: embT/wihe are the first big DMAs to land
            for mc in range(MG):
                p = sps.tile([128, NT], F32, tag="p_gie", bufs=2)
                for kc in range(KD):
                    nc.tensor.matmul(
                        p[:],
                        wihe[:, kc * 3 * DEC + mc * 128: kc * 3 * DEC + mc * 128 + 128],
                        embT[:, kc * NT: (kc + 1) * NT],
                        start=(kc == 0), stop=(kc == KD - 1))
                if mc % 2 == 0:
                    nc.vector.tensor_scalar(
                        gi_emb[:, mc * NT: (mc + 1) * NT], p[:],
                        biasgi[:, mc: mc + 1], None, Op.add)
                else:
                    nc.scalar.activation(
                        gi_emb[:, mc * NT: (mc + 1) * NT], p[:],
                        AF.Identity, bias=biasgi[:, mc: mc + 1])

            # enc = W_feat.T @ spatialT (+ b_feat), feature-major
            p_encs = [sps.tile([128, NL], F32, tag=f"p_enc{mc}", name=f"p_enc{mc}", bufs=1)
                      for mc in range(KD)]
            for kc in range(KE):
                for mc in range(KD):
                    nc.tensor.matmul(
                        p_encs[mc][:],
                        wfeat[:, kc * DEC + mc * 128: kc * DEC + mc * 128 + 128],
                        spatialT[:, kc * NL: (kc + 1) * NL],
                        start=(kc == 0), stop=(kc == KE - 1))
            for mc in range(KD):
                if mc % 2 == 0:
                    nc.vector.tensor_scalar(
                        enc_fm[:, mc * NL: (mc + 1) * NL], p_encs[mc][:],
                        bfeat[:, mc: mc + 1], None, Op.add)
                else:
                    nc.scalar.activation(
                        enc_fm[:, mc * NL: (mc + 1) * NL], p_encs[mc][:],
                        AF.Identity, bias=bfeat[:, mc: mc + 1])

            # att1 = W_ea.T @ enc (+ b_ea + b_da)  -> bf16 [att-chunk, b*49+l]
            for mc in range(MA):
                p = sps.tile([128, NL], F32, tag="p_att1", bufs=2)
                for kc in range(KD):
                    nc.tensor.matmul(
                        p[:],
                        wea[:, kc * ATT + mc * 128: kc * ATT + mc * 128 + 128],
                        enc_fm[:, kc * NL: (kc + 1) * NL],
                        start=(kc == 0), stop=(kc == KD - 1))
                if mc % 2 == 0:
                    nc.vector.tensor_scalar(
                        att1[:, mc * NL: (mc + 1) * NL], p[:],
                        bea[:, mc: mc + 1], None, Op.add)
                else:
                    nc.scalar.activation(
                        att1[:, mc * NL: (mc + 1) * NL], p[:],
                        AF.Identity, bias=bea[:, mc: mc + 1])

            ctx_sps.__exit__(None, None, None)
            sps2 = ctx_sps2 = tc.tile_pool(name="sps2", bufs=1, space="PSUM")
            sps2 = sps2.__enter__()
            # enc_b[b]: rows 0:49 = enc[b] as [l, d]: 4 transpose matmuls into
            # one psum bank per b, then a single wide copy (DVE/ACT alternating)
            for b in range(BL):
                pt = sps2.tile([128, DEC], F32, tag="p_tr", bufs=4)
                for c in range(KD):
                    nc.tensor.matmul(
                        pt[0:L, c * 128: (c + 1) * 128],
                        enc_fm[:, c * NL + b * L: c * NL + b * L + L],
                        ident_f[:], start=True, stop=True)
                if b % 2 == 0:
                    nc.vector.tensor_copy(enc_b[b][0:L, :], pt[0:L, :])
                else:
                    nc.scalar.activation(enc_b[b][0:L, :], pt[0:L, :], AF.Identity)


            ctx_sps2.__exit__(None, None, None)

        # ------------------------------ recurrence ------------------------------
        def fc_mms(s, fps):
            """logitsT for timestep s from hist col block s+1; vb-major psum."""
            p_fcs = [fps.tile([128, VBA * BL], F32, tag="p_fcA", name="p_fcA"),
                     fps.tile([128, (VB - VBA) * BL], F32, tag="p_fcB", name="p_fcB")]
            for vb in range(VB):
                half = 0 if vb < VBA else 1
                col = (vb - (0 if vb < VBA else VBA)) * BL
                nv = 128 if vb < VB - 1 else V - (VB - 1) * 128
                for kc in range(KD):
                    nc.tensor.matmul(
                        p_fcs[half][0:nv, col: col + BL],
                        wfc_sb[:, kc * V + vb * 128: kc * V + vb * 128 + nv],
                        hist[:, 32 * (s + 1) + 8 * kc: 32 * (s + 1) + 8 * kc + 8],
                        start=(kc == 0), stop=(kc == KD - 1))
            return p_fcs

        _NOSYNC = mybir.DependencyInfo(sync=False, no_sync=True)

        def fc_out(s, p_fcs, rsb, after_dve=None, after_act=None):
            fcsb = rsb.tile([128, FCW], BF16, tag="fcsb", bufs=3)
            cpA = nc.vector.tensor_copy(fcsb[:, 0: VBA * BL], p_fcs[0][:])
            cpB = nc.scalar.copy(fcsb[:, VBA * BL: FCW], p_fcs[1][:])
            # keep the staging copies out of the attention chain: same-engine
            # ordering edges (no runtime sems) behind the 2nd add / the exp
            if after_dve is not None:
                cpA.ins.add_dependency(after_dve.ins.name, _NOSYNC)
            if after_act is not None:
                cpB.ins.add_dependency(after_act.ins.name, _NOSYNC)
            nc.sync.dma_start(d_logits[:, s * FCW: (s + 1) * FCW], fcsb[:])

        with tc.tile_pool(name="rsb", bufs=3) as rsb, \
             tc.tile_pool(name="rps", bufs=1, space="PSUM") as rps, \
             tc.tile_pool(name="fps", bufs=2, space="PSUM") as fps:
            for t in range(n_steps):
                hprev = [hist[:, 32 * t + 8 * kc: 32 * t + 8 * kc + 8] for kc in range(KD)]
                gie = gi_emb[:].rearrange("p (mc tb) -> p mc tb", mc=MG)

                # fc for the lagged step first: its matmuls have no
                # dependencies on this step, so they execute in the PE idle
                # window while the previous step's gate chain runs on DVE/ACT.
                # (the psum->SBUF copies are issued after the attention adds
                # so they don't interleave into the DVE/ACT chain)
                p_fcs = fc_mms(t - LAG, fps) if t >= LAG else None

                # One packed psum bank for the small per-step accumulators:
                # att2 0:16 | sc 16:24 | small 24:32 | ctx 32:64 | gin 64:96 | ghn 96:128
                p_mix = rps.tile([128, 128], F32, tag="p_mix")

                for mc in range(MA):
                    for kc in range(KD):
                        nc.tensor.matmul(
                            p_mix[:, mc * 8: mc * 8 + 8],
                            wda[:, (kc * MA + mc) * 128: (kc * MA + mc) * 128 + 128],
                            hprev[kc], start=(kc == 0), stop=(kc == KD - 1))

                # gh (r,z and n): gi_emb slice and 0.5*b_hh_n enter psum via
                # identity matmuls; ACT reads gates straight from psum.
                p_ghrz = rps.tile([128, 64], F32, tag="p_ghrz", bufs=2)
                nc.tensor.matmul(
                    p_ghrz[:], ident_f[:], gie[:, 0:8, 8 * t: 8 * t + 8],
                    start=True, stop=False)
                nc.tensor.matmul(
                    p_mix[:, 96:128], ident_f[:], bhhnbc[:], start=True, stop=False)
                for mc in range(8):
                    for kc in range(KD):
                        nc.tensor.matmul(
                            p_ghrz[:, mc * 8: mc * 8 + 8],
                            whh[:, (kc * MG + mc) * 128: (kc * MG + mc) * 128 + 128],
                            hprev[kc], start=False, stop=False)
                for mc in range(8, MG):
                    for kc in range(KD):
                        nc.tensor.matmul(
                            p_mix[:, 96 + (mc - 8) * 8: 96 + (mc - 8) * 8 + 8],
                            whh[:, (kc * MG + mc) * 128: (kc * MG + mc) * 128 + 128],
                            hprev[kc], start=False,
                            stop=(kc == KD - 1 and mc == MG - 1))

                # att2 bf16 copy for Pool (Pool cannot read PSUM); chunk-0 add
                # on DVE straight from psum, chunk-1 add on Pool, so the two
                # adds run concurrently and tanh c1 is not gated on DVE.
                for c in range(MA):
                    i_add = nc.vector.tensor_tensor(
                        targ[:, c * NL: (c + 1) * NL].rearrange(
                            "p (b l) -> p b l", b=BL, l=L),
                        att1[:, c * NL: (c + 1) * NL].rearrange(
                            "p (b l) -> p b l", b=BL, l=L),
                        p_mix[:, c * BL: (c + 1) * BL].unsqueeze(2)
                        .broadcast_to([128, BL, L]),
                        Op.add)
                    nc.scalar.activation(
                        tanh_sb[:, c * NL: (c + 1) * NL],
                        targ[:, c * NL: (c + 1) * NL], AF.Tanh)

                # scores -> psum [l rows 0:49, b cols]. b-outer: each column's
                # accumulation group is consecutive — PSUM groups are bank-
                # scoped, so interleaved open groups in one bank lose updates.
                prev_sc = None
                for b in range(BL):
                    for kc in range(MA):
                        i_sc = nc.tensor.matmul(
                            p_mix[0:L, 16 + b: 16 + b + 1],
                            tanh_sb[:, kc * NL + b * L: kc * NL + b * L + L],
                            wfa[:, kc: kc + 1],
                            start=(kc == 0), stop=(kc == MA - 1))
                        if prev_sc is not None:
                            i_sc.ins.add_dependency(prev_sc.ins.name, _NOSYNC)
                        prev_sc = i_sc

                # single exp (no max-subtraction: scores are tiny)
                i_exp = nc.scalar.activation(expe[0:L, 0:BL], p_mix[0:L, 16:24], AF.Exp)
                if p_fcs is not None:
                    fc_out(t - LAG, p_fcs, rsb, after_dve=i_add, after_act=i_exp)

                # denominator broadcast to all partitions in one matmul
                # (lhsT = ones [49, 128]); context matmuls run concurrently
                nc.tensor.matmul(p_mix[:, 24:32], ones_mat_bf[0:L, :], expe[0:L, :],
                                 start=True, stop=True)
                for b in range(BL):
                    for c in range(KD):
                        nc.tensor.matmul(
                            p_mix[:, 32 + c * 8 + b: 32 + c * 8 + b + 1],
                            enc_b[b][0:L, c * 128: (c + 1) * 128],
                            expe[0:L, b: b + 1],
                            start=True, stop=True)
                rb_sb = rsb.tile([128, BL], F32, tag="rb_sb")
                nc.vector.reciprocal(rb_sb[:], p_mix[:, 24:32])
                x_ctx = rsb.tile([128, KD * BL], BF16, tag="x_ctx")
                nc.vector.tensor_tensor(
                    x_ctx[:].rearrange("p (c b) -> p c b", c=KD),
                    p_mix[:, 32:64].rearrange("p (c b) -> p c b", c=KD),
                    rb_sb[:].unsqueeze(1).broadcast_to([128, KD, BL]),
                    Op.mult)

                # gi_ctx: r,z accumulate onto p_ghrz; n into p_gin (pre-loaded
                # with the gi_emb n-slice via identity matmul)
                nc.tensor.matmul(
                    p_mix[:, 64:96], ident_f[:], gie[:, 8:MG, 8 * t: 8 * t + 8],
                    start=True, stop=False)
                for mc in range(8):
                    for kc in range(KD):
                        nc.tensor.matmul(
                            p_ghrz[:, mc * 8: mc * 8 + 8],
                            wihc[:, (kc * MG + mc) * 128: (kc * MG + mc) * 128 + 128],
                            x_ctx[:, kc * 8: kc * 8 + 8], start=False,
                            stop=(kc == KD - 1 and mc == 7))
                for mc in range(8, MG):
                    for kc in range(KD):
                        nc.tensor.matmul(
                            p_mix[:, 64 + (mc - 8) * 8: 64 + (mc - 8) * 8 + 8],
                            wihc[:, (kc * MG + mc) * 128: (kc * MG + mc) * 128 + 128],
                            x_ctx[:, kc * 8: kc * 8 + 8], start=False,
                            stop=(kc == KD - 1 and mc == MG - 1))

                # gates: t_rz = tanh(0.5 * rz_full) straight from psum;
                # sigmoids are 0.5*t+0.5. The n-chain is
                #   vv = r'*gh_n = (t_r + 1) * (0.5*gh_n)   [one fused STT;
                #        the 0.5 is folded into whh's n-block host-side]
                #   n = tanh(vv + gi_n)
                # and issues on DVE before the off-chain z ops, which then
                # execute under the ACT tanh.
                t_rz = rsb.tile([128, 64], F32, tag="t_rz")
                nc.scalar.activation(t_rz[:], p_ghrz[:], AF.Tanh, scale=0.5)
                vv = rsb.tile([128, 32], F32, tag="vv")
                nc.vector.scalar_tensor_tensor(
                    vv[:], t_rz[:, 0:32], 1.0, p_mix[:, 96:128], Op.add, Op.mult)
                n_arg = rsb.tile([128, 32], F32, tag="n_arg")
                i_narg = nc.vector.tensor_tensor(n_arg[:], vv[:], p_mix[:, 64:96], Op.add)
                n_g = rsb.tile([128, 32], F32, tag="n_g")
                nc.scalar.activation(n_g[:], n_arg[:], AF.Tanh)
                # z-branch ops execute on DVE under the ACT tanh; keep them
                # behind n_arg so they don't lengthen the n-gate chain
                zm = rsb.tile([128, 32], F32, tag="zm")
                i_zm = nc.vector.tensor_scalar(zm[:], t_rz[:, 32:64], -0.5, 0.5, Op.mult, Op.add)
                i_zm.ins.add_dependency(i_narg.ins.name, _NOSYNC)
                trz1_z = rsb.tile([128, 32], F32, tag="trz1_z")
                nc.vector.tensor_scalar(trz1_z[:], t_rz[:, 32:64], 0.5, 0.5, Op.mult, Op.add)
                w1 = rsb.tile([128, 32], F32, tag="w1")
                nc.vector.tensor_tensor(
                    w1[:], hist[:, 32 * t: 32 * t + 32], trz1_z[:], Op.mult)
                # h_new = n*(1-z') + h*z' -> written straight into bf16 history
                u_g = rsb.tile([128, 32], F32, tag="u_g")
                nc.vector.tensor_tensor(u_g[:], n_g[:], zm[:], Op.mult)
                nc.vector.tensor_tensor(
                    hist[:, 32 * (t + 1): 32 * (t + 1) + 32], u_g[:], w1[:], Op.add)

            # fc tail for the last LAG steps
            for s in range(max(0, n_steps - LAG), n_steps):
                fc_out(s, fc_mms(s, fps), rsb)
            if dump_hist:
                nc.sync.dma_start(d_histd[:], hist[:])
            if dump_attn:
                nc.sync.dma_start(d_targd[:], targ[:])
                nc.sync.dma_start(d_tanhd[:], tanh_sb[:])
                nc.sync.dma_start(d_exped[:], expe[:])
                nc.sync.dma_start(d_xctxd[:], x_ctx[:])
                nc.sync.dma_start(d_att1d[:], att1[:])

    nc.finalize()
    return nc


# ------------------------------ host-side prep ------------------------------

def _chunk_lhs(w, k):
    """[K, M] -> [128, (K/128)*M] with col = kc*M + m."""
    K, M = w.shape
    return np.ascontiguousarray(w.reshape(k, 128, M).transpose(1, 0, 2).reshape(128, k * M))


def _chunk_lhs_sq(w, k, mchunks):
    """[K, M] -> [128, k*mchunks*128] with col = (kc*mchunks+mc)*128 + j."""
    K, M = w.shape
    return np.ascontiguousarray(
        w.reshape(k, 128, mchunks, 128).transpose(1, 0, 2, 3).reshape(128, k * mchunks * 128))


def _bf(x):
    return np.ascontiguousarray(x.astype(ml_dtypes.bfloat16))


def host_prep(inputs, n_steps=T):
    i = {k: np.asarray(v) for k, v in inputs.items()}
    sf = i["spatial_feats"].astype(np.float32)          # [64, 49, 2048]
    cap = i["captions"].astype(np.int64)                # [64, 50]
    W_feat, b_feat = i["W_feat"].astype(np.float32), i["b_feat"].astype(np.float32)
    W_ea, b_ea = i["W_ea"].astype(np.float32), i["b_ea"].astype(np.float32)
    W_da, b_da = i["W_da"].astype(np.float32), i["b_da"].astype(np.float32)
    W_fa = i["W_fa"].astype(np.float32)
    emb = i["emb"].astype(np.float32)
    W_ih, W_hh = i["W_ih"].astype(np.float32), i["W_hh"].astype(np.float32)
    b_ih, b_hh = i["b_ih"].astype(np.float32), i["b_hh"].astype(np.float32)
    W_fc = i["W_fc"].astype(np.float32)

    # scale the n-block of W_hh / b_hh_n by 0.5: the fused gate op computes
    # r'*gh_n as (tanh+1)*(0.5*gh_n)
    W_hhT = np.ascontiguousarray(W_hh.T).copy()
    W_hhT[:, 2 * DEC:] *= 0.5

    shared = {
        "wfeat": _bf(_chunk_lhs(W_feat, KE)),
        "wea": _bf(_chunk_lhs(W_ea, KD)),
        "wihe": _bf(_chunk_lhs(np.ascontiguousarray(W_ih[:, :EMB].T), KD)),
        "wihc": _bf(_chunk_lhs_sq(np.ascontiguousarray(W_ih[:, EMB:].T), KD, MG)),
        "whh": _bf(_chunk_lhs_sq(W_hhT, KD, MG)),
        "wda": _bf(_chunk_lhs_sq(W_da, KD, MA)),
        "wfa": _bf(W_fa.reshape(MA, 128).T),
        "wfc": _bf(W_fc.reshape(KD, 128, V).transpose(1, 0, 2).reshape(128, KD * V)),
        "bfeat": np.ascontiguousarray(b_feat.reshape(KD, 128).T),
        "bea": np.ascontiguousarray((b_ea + b_da).reshape(MA, 128).T),
        "biasgi": np.ascontiguousarray(
            (b_ih + np.concatenate([b_hh[:2 * DEC], np.zeros(DEC, np.float32)])).reshape(MG, 128).T),
        "bhhnbc": _bf(
            np.repeat((0.5 * b_hh[2 * DEC:]).reshape(4, 128).T[:, :, None], BL, axis=2).reshape(128, 4 * BL)),
    }
    in_maps = []
    for c in range(NCORES):
        sl = slice(c * BL, (c + 1) * BL)
        sfT = sf[sl].reshape(NL, ENC).T                      # [2048, 392]
        embs = emb[cap[sl][:, :n_steps]]                     # [8, n_steps, 512]
        embT = embs.transpose(1, 0, 2).reshape(BL * n_steps, EMB).T   # [512, NT]
        m = dict(shared)
        m["spatialT"] = _bf(sfT.reshape(KE, 128, NL).transpose(1, 0, 2).reshape(128, KE * NL))
        m["embT"] = _bf(embT.reshape(KD, 128, BL * n_steps).transpose(1, 0, 2).reshape(128, KD * BL * n_steps))
        in_maps.append(m)
    return in_maps


def unshard(results, b_fc, n_steps=T):
    outs = []
    for c in range(NCORES):
        lg = results[c]["logits"]                          # [128, T*FCW] bf16
        lg = np.asarray(lg).reshape(128, n_steps, VB, BL)
        lg = lg.transpose(3, 1, 2, 0).reshape(BL, n_steps, VB * 128)[:, :, :V]
        outs.append(lg)
    out = np.concatenate(outs, axis=0).astype(np.float32)    # [64, T, 10000]
    out += np.asarray(b_fc).astype(np.float32)[None, None, :]
    return out


_PROG_CACHE = {}


def _get_prog(n_steps=T):
    if n_steps not in _PROG_CACHE:
        _PROG_CACHE[n_steps] = build_program(n_steps)
    return _PROG_CACHE[n_steps]


def kernel(**inputs):
    from concourse.bass_utils import run_bass_kernel_spmd
    nc = _get_prog(T)
    in_maps = host_prep(inputs, T)
    try:
        res = run_bass_kernel_spmd(nc, in_maps, core_ids=list(range(NCORES)))
    except Exception:
        # transient device errors (e.g. NRT_EXEC_UNIT_UNRECOVERABLE from a
        # previously wedged core) usually clear on retry
        res = run_bass_kernel_spmd(nc, in_maps, core_ids=list(range(NCORES)))
    return unshard(res.results, inputs["b_fc"], T)
